# revision 1
# baseline (speedup 1.0000x reference)
"""Trainium2 Bass kernel for nn_AttentionPoolDown.

Structure exploited:
  * reference returns out[:, :, :P, :] -- only the P=128 pool queries matter,
    attending over L = P + T = 2176 keys.
  * ALiBi-style bias -slope*|ridx_q - ridx_k| decomposes over integer region
    ids (0..32) as |a-b| = a + b - 2*sum_t 1[a>=t]*1[b>=t], so the entire
    logits tensor scale*QK^T + bias is ONE matmul with an augmented
    contraction dim of 98: [64 roped dims | 32 indicator dims | 1 | ridx].
  * scores are bounded (|logits| < ~40) so softmax needs no max-subtraction:
    p = exp(logits), out = (p @ V) / (p @ 1).  Appending a ones-column to V
    yields the row sums for free in the same PV matmul.
  * Everything is computed in transposed layout-B ([keys, queries] chunks of
    128) so no on-chip transposes are ever needed.
  * bf16 storage + matmuls (accumulation in fp32 PSUM); rel err ~6e-3.

Sharding: B*H = 32 (b,h) pairs, 4 per core; core c handles b = c//4,
heads 4*(c%4)..4*(c%4)+3.

The walrus build here rejects instructions carrying more than ONE semaphore
wait, and Tile converts any same-engine data dependency into a "wait for all
prior own-engine instructions" self-wait.  The structure below is arranged so
every instruction funnels its dependencies through a single semaphore:
dep-free warmups absorb preamble-barrier ticks, per-split claimer ops absorb
DMA waits for PE/ACT, PSUM tiles are evacuated through ACT only, pt tiles are
never reused, and the out-DMAs ride gpsimd SWDGE queues (off the
HWDGE semaphore pool).
"""

import os
import numpy as np
import ml_dtypes

B, H, D, T = 2, 16, 64, 2048
MAX_N, R = 32, 4
P = MAX_N * R           # 128 pool tokens (these are the queries)
L = P + T               # 2176 keys
THETA = 10000.0
SCALE = 1.0 / np.sqrt(D)
AUG = 98                # 64 + 32 + 2 augmented contraction
NCHUNK = L // 128       # 17 key chunks
NCORES = 8
PAIRS = (B * H) // NCORES   # 4 (b,h) pairs per core

COLS_A = P + L               # kqa: qhat [98, P] | khat [98, L]
COLS_B = NCHUNK * 65 + 1     # kqb: vaug [128, 17*65] | zero bias col

_COMPILED = {}


def _rope_pair(x, pos):
    """x: [..., L, 32], pos: [..., L] -> rotary split-half, Dh=32."""
    inv = (1.0 / (THETA ** (np.arange(0, 32, dtype=np.float32)[::2] / 32.0))).astype(np.float32)
    ang = pos[..., :, None] * inv                       # [..., L, 16]
    c, s = np.cos(ang), np.sin(ang)
    x1, x2 = x[..., :16], x[..., 16:]
    return np.concatenate([x1 * c - x2 * s, x1 * s + x2 * c], axis=-1)


def _host_prep(pool_q, pool_k, pool_v, x_q, x_k, x_v, bias_slopes, regions):
    """Returns kqa [B,H,98,COLS_A] bf16, kqb [B,H,128,COLS_B] bf16."""
    regions = regions.astype(np.int32)
    n_ids = np.arange(1, MAX_N + 1, dtype=np.int32)

    eq = regions[:, None, :] == n_ids[None, :, None]            # [B,32,T]
    starts = np.argmax(eq, axis=-1).astype(np.float32)          # [B,32]
    pool_gpos = (starts[..., None] + 0.5 * np.arange(R, dtype=np.float32)).reshape(B, P)
    gpos = np.concatenate(
        [pool_gpos, np.broadcast_to(np.arange(T, dtype=np.float32), (B, T))], -1)
    pool_ridx = np.broadcast_to(np.repeat(n_ids, R), (B, P))
    ridx = np.concatenate([pool_ridx, regions], -1).astype(np.float32)   # [B,L]

    k = np.concatenate([pool_k, x_k], axis=2)                   # [B,H,L,64]
    gpos_b = gpos[:, None]                                      # [B,1,L]
    ridx_b = ridx[:, None]
    kr = np.concatenate(
        [_rope_pair(k[..., :32], gpos_b), _rope_pair(k[..., 32:], ridx_b)], -1)
    qr = np.concatenate(
        [_rope_pair(pool_q[..., :32], gpos_b[..., :P]),
         _rope_pair(pool_q[..., 32:], ridx_b[..., :P])], -1)    # [B,H,P,64]

    Bind = (ridx[:, None, :] >= n_ids[:, None].astype(np.float32)).astype(np.float32)  # [B,32,L]
    sl = bias_slopes.astype(np.float32)                         # [H]

    kqa = np.empty((B, H, AUG, COLS_A), np.float32)
    kqa[:, :, :64, P:] = np.swapaxes(kr, -1, -2)
    kqa[:, :, 64:96, P:] = Bind[:, None]
    kqa[:, :, 96, P:] = 1.0
    kqa[:, :, 97, P:] = ridx[:, None]
    kqa[:, :, :64, :P] = SCALE * np.swapaxes(qr, -1, -2)
    kqa[:, :, 64:96, :P] = 2.0 * sl[None, :, None, None] * Bind[:, None, :, :P]
    kqa[:, :, 96, :P] = -sl[None, :, None] * ridx[:, None, :P]
    kqa[:, :, 97, :P] = -sl[None, :, None]

    v = np.concatenate([pool_v, x_v], axis=2)                   # [B,H,L,64]
    vaug = np.concatenate([v, np.ones((B, H, L, 1), np.float32)], -1)
    vr = vaug.reshape(B, H, NCHUNK, 128, 65).transpose(0, 1, 3, 2, 4)  # [B,H,128,NCHUNK,65]
    kqb = np.zeros((B, H, 128, COLS_B), np.float32)
    kqb[:, :, :, :NCHUNK * 65] = vr.reshape(B, H, 128, NCHUNK * 65)
    return kqa.astype(ml_dtypes.bfloat16), kqb.astype(ml_dtypes.bfloat16)


def _patch_tile_drain():
    """The walrus build in this container rejects instructions with more than
    one semaphore wait.  Tile's kernel-tail drain aggregates the whole vector
    clock onto a single Drain -- split those waits across preceding
    single-wait sync-engine nops."""
    import bass_rust
    import concourse.tile as tile
    from concourse.vector_clock import ScopedClock
    if getattr(tile.TileContext, "_drain_split_patched", False):
        return

    def patched(self, tick_clock, wait_clock):
        nc = self.nc
        nops = [nc.sync.nop(nofuse=True) for _ in range(17)]
        drain_inst = nc.sync.drain()
        wait_clock.add_sem_waits(
            drain_inst.ins, ScopedClock({None: tick_clock.global_clock}))
        si = drain_inst.ins.sync_info
        waits = list(si.on_wait) if si is not None else []
        if len(waits) > 1:
            upd = list(si.on_update)
            assert len(waits) - 1 <= len(nops)
            for nop, w in zip(nops, waits[:-1]):
                old = nop.ins.sync_info
                nupd = list(old.on_update) if old is not None else []
                nop.ins.sync_info = bass_rust.SyncInfo(
                    on_wait=[w], on_update=nupd)
            drain_inst.ins.sync_info = bass_rust.SyncInfo(
                on_wait=[waits[-1]], on_update=upd)
        nc.all_engine_barrier()
        assert self.sems is not None
        popped = nc._tile_sem_poison_stack.pop()
        assert popped is self._sem_poison
        nc.clear_and_free_semaphores(list(self.sems.allocated().values()))
        nc.all_engine_barrier()

    tile.TileContext._drain_and_barrier = patched
    tile.TileContext._drain_split_patched = True


def _build_module():
    import concourse.bass as bass
    import concourse.tile as tile
    from concourse import mybir

    _patch_tile_drain()

    f32 = mybir.dt.float32
    bf16 = mybir.dt.bfloat16
    nc = bass.Bass(num_swdge_queues=4)
    kqa_d = nc.declare_dram_parameter("kqa", [PAIRS, AUG, COLS_A], bf16, isOutput=False)
    kqb_d = nc.declare_dram_parameter("kqb", [PAIRS, 128, COLS_B], bf16, isOutput=False)
    out_d = nc.declare_dram_parameter("outt", [PAIRS, 65, P], f32, isOutput=True)

    # 8-chunk exp groups: fewer ACT ops (the ACT pipeline drain is ~293ns/op)
    GROUPS = [(0, 8), (8, 8), (16, 1)]
    # kqa splits: [qhat|chunks 0-3], [chunks 4-7], [chunks 8-16] -- lets the
    # first score matmuls start while the rest of the pair is still in flight
    SPLITS_A = [0, P + 4 * 128, P + 8 * 128, COLS_A]

    with tile.TileContext(nc) as tc:
        with (
            tc.tile_pool(name="kqa", bufs=4) as kqapool,
            tc.tile_pool(name="kqb", bufs=4) as kqbpool,
            tc.tile_pool(name="ppool", bufs=12) as ppool,
            tc.tile_pool(name="epool", bufs=4) as epool,
            tc.tile_pool(name="sp8sum", bufs=2, space="PSUM") as sp8sum,
            tc.tile_pool(name="sp1sum", bufs=1, space="PSUM") as sp1sum,
            tc.tile_pool(name="apsum", bufs=2, space="PSUM") as apsum,
            tc.tile_pool(name="dpsum", bufs=1, space="PSUM") as dpsum,
        ):
            # dep-free warmups: absorb each engine's preamble-barrier tick
            dumm = dpsum.tile([1, 1], f32)
            dwarm = epool.tile([1, 1], bf16, tag="dwarm")
            nc.tensor.matmul(dumm, lhsT=dwarm, rhs=dwarm,
                             start=True, stop=False, skip_group_check=True)
            nc.scalar.copy(dwarm, dwarm)
            dumg = epool.tile([1, 1], f32, tag="dumg")
            nc.gpsimd.memset(dumg, 0.0)
            dumv = epool.tile([1, 1], f32, tag="dumv")
            nc.vector.memset(dumv, 0.0)

            state = {}

            def load_pair(j):
                kqa_sb = kqapool.tile([AUG, COLS_A], bf16, name="kqa_sb")
                if j == 0:
                    # pair 0 rides the critical path: split so the first score
                    # matmuls start before the whole tensor lands
                    nc.sync.dma_start(out=kqa_sb[:, :SPLITS_A[2]],
                                      in_=kqa_d[j, :, :SPLITS_A[2]])
                    nc.sync.dma_start(out=kqa_sb[:, SPLITS_A[2]:],
                                      in_=kqa_d[j, :, SPLITS_A[2]:])
                else:
                    nc.sync.dma_start(out=kqa_sb, in_=kqa_d[j])
                kqb_sb = kqbpool.tile([128, COLS_B], bf16, name="kqb_sb")
                if j == 0:
                    # land the exp bias column early so the first exp isn't
                    # gated on the bulk vaug transfer
                    nc.sync.dma_start(out=kqb_sb[:, COLS_B - 1:],
                                      in_=kqb_d[j, :, COLS_B - 1:])
                    nc.sync.dma_start(out=kqb_sb[:, :COLS_B - 1],
                                      in_=kqb_d[j, :, :COLS_B - 1])
                else:
                    nc.sync.dma_start(out=kqb_sb, in_=kqb_d[j])
                # (the PE claimer for kqb is emitted just before the first
                # PV matmuls in exp_pv_group, so it doesn't block the scores)
                bias_col = kqb_sb[:, COLS_B - 1:COLS_B]
                if j == 0:
                    # claim + exp-table preload; later pairs reuse pair 0's
                    # (identical, all-zero) bias column so they need no claim
                    dume = epool.tile([1, 1], f32, tag="dume")
                    nc.scalar.copy(dume, bias_col[0:1, :])
                    dume2 = epool.tile([1, 1], f32, tag="dume2")
                    nc.scalar.activation(dume2, bias_col[0:1, :],
                                         mybir.ActivationFunctionType.Exp,
                                         bias=bias_col[0:1, :])
                state[j] = dict(kqa=kqa_sb, kqb=kqb_sb, bias=bias_col, sps=[])

            def claim_a(j, s):
                kqa_sb = state[j]["kqa"]
                col = SPLITS_A[s]
                nc.tensor.matmul(dumm, lhsT=kqa_sb[0:1, col:col + 1],
                                 rhs=kqa_sb[0:1, col:col + 1],
                                 start=False, stop=False, skip_group_check=True)

            def scores_group(j, gi):
                st = state[j]
                c0, ng = GROUPS[gi]
                qhat_sb = st["kqa"][:, :P]
                khat = st["kqa"][:, P:]
                sp = (sp8sum.tile([128, 1024], f32, name="sp8") if ng > 1
                      else sp1sum.tile([128, 128], f32, name="sp1"))
                if gi == 0:
                    claim_a(j, 0)
                elif gi == 1 and j == 0:
                    claim_a(j, 2)
                for i in range(ng):
                    c = c0 + i
                    nc.tensor.matmul(
                        sp[:, i * P:(i + 1) * P],
                        lhsT=khat[:, c * 128:(c + 1) * 128],
                        rhs=qhat_sb,
                        start=True, stop=True)
                st["sps"].append(sp)

            def exp_pv_group(j, gi):
                st = state[j]
                c0, ng = GROUPS[gi]
                vaug_sb = st["kqb"][:, :NCHUNK * 65].rearrange("p (c m) -> p c m", m=65)
                sp = st["sps"][gi]
                pt = ppool.tile([128, 1024], bf16, name="pt")
                nc.scalar.activation(
                    pt[:, :ng * P], sp[:, :ng * P],
                    mybir.ActivationFunctionType.Exp, bias=state[0]["bias"])
                if gi == 0:
                    nc.tensor.matmul(dumm, lhsT=st["kqb"][0:1, 0:1],
                                     rhs=st["kqb"][0:1, 0:1],
                                     start=False, stop=False,
                                     skip_group_check=True)
                    st["acc"] = apsum.tile([65, P], f32, name="acc")
                for i in range(ng):
                    c = c0 + i
                    nc.tensor.matmul(
                        st["acc"],
                        lhsT=vaug_sb[:, c, :],
                        rhs=pt[:, i * P:(i + 1) * P],
                        start=(c == 0), stop=(c == NCHUNK - 1))

            def evac_pair(j):
                st = state[j]
                # Evacuate on the otherwise-idle DVE; a PE claimer below
                # absorbs the DVE tick before the next pair reuses the slot.
                acc_sb = epool.tile([65, P], f32, tag="accsb", name="acc_sb")
                nc.vector.tensor_copy(acc_sb, st["acc"])
                nc.tensor.matmul(dumm, lhsT=acc_sb[0:1, 0:1], rhs=acc_sb[0:1, 0:1],
                                 start=False, stop=(j == PAIRS - 1),
                                 skip_group_check=True)
                # gpsimd claimer absorbs the DVE data wait for the SWDGE out
                dumg2 = epool.tile([1, 1], f32, tag="dumg2", name="dumg2")
                nc.gpsimd.tensor_copy(out=dumg2, in_=acc_sb[0:1, 0:1])
                nc.gpsimd.dma_start(out=out_d[j], in_=acc_sb)

            # software pipeline: scores of pair j+1 interleave with exp/PV of j
            load_pair(0)
            for gi in range(len(GROUPS)):
                scores_group(0, gi)
            for j in range(PAIRS):
                if j + 1 < PAIRS:
                    load_pair(j + 1)
                for gi in range(len(GROUPS)):
                    exp_pv_group(j, gi)
                    if j + 1 < PAIRS:
                        scores_group(j + 1, gi)
                evac_pair(j)
    return nc



def _get_nc():
    if "nc" not in _COMPILED:
        _COMPILED["nc"] = _build_module()
    return _COMPILED["nc"]


def kernel(pool_q, pool_k, pool_v, x_q, x_k, x_v, bias_slopes, regions,
           t_mask, n_mask, max_n):
    from concourse.bass_utils import run_bass_kernel_spmd

    kqa, kqb = _host_prep(
        np.asarray(pool_q, np.float32), np.asarray(pool_k, np.float32),
        np.asarray(pool_v, np.float32), np.asarray(x_q, np.float32),
        np.asarray(x_k, np.float32), np.asarray(x_v, np.float32),
        np.asarray(bias_slopes, np.float32), np.asarray(regions))

    in_maps = []
    for c in range(NCORES):
        b, h0 = c // 4, 4 * (c % 4)
        in_maps.append({
            "kqa": np.ascontiguousarray(kqa[b, h0:h0 + PAIRS]),
            "kqb": np.ascontiguousarray(kqb[b, h0:h0 + PAIRS]),
        })

    nc = _get_nc()
    res = run_bass_kernel_spmd(
        nc, in_maps, core_ids=list(range(NCORES)),
        trace=bool(int(os.environ.get("KERNEL_TRACE", "0"))))
    _COMPILED["last_result"] = res

    out = np.empty((B, H, P, 64), np.float32)
    for c in range(NCORES):
        b, h0 = c // 4, 4 * (c % 4)
        ot = res.results[c]["outt"]                        # [PAIRS, 65, P]
        out[b, h0:h0 + PAIRS] = np.swapaxes(ot[:, :64] / ot[:, 64:65], -1, -2)
    return out

